# revision 8
# baseline (speedup 1.0000x reference)
"""DissipativeThetaRINN Trainium2 (Bass/Tile) kernel — 8-core data parallel.

Strategy (pure data parallel, per sharding hint):
  - Batch B=2048 is split across 8 NeuronCores (256 rows/core); the tiny
    controller matrices and value-MLP weights are replicated.
  - On-device layout is transposed: features on SBUF partitions, batch on
    the free dimension.
  - Per timestep the implicit layer w = tanh(Cv x + Dvy y + Dvw w) is run
    as a fixed-point iteration. The batch is split into two 128-column
    chunks so chunk A's tanh (ScalarE) overlaps chunk B's matmuls (PE).
    The constant term is re-folded into PSUM by a second accumulating
    matmul each iteration, so ScalarE only does one Tanh per chunk.
  - The fixed point contracts with factor ~0.47/iter; N_ITERS iterations
    reproduce the reference's 30-iteration result to ~1e-4 (the
    reference's own iterate converges to fp32 noise by ~iteration 20).
  - Matmuls run in fp16 (PSUM accumulates fp32); the x recurrence keeps an
    fp32 accumulator on device, and DT is pre-folded into the recurrence
    weights so fp16 rounding only touches the 0.01-scaled increment.
  - The value MLP (independent of the recurrence) is computed in grouped
    timestep pairs and scheduled into the fixed-point loop's engine gaps.
  - log_stds broadcast and the +b2 value bias are applied host-side during
    output assembly.
"""
import numpy as np
import concourse.bass as bass
import concourse.mybir as mybir
import concourse.tile as tile
from concourse import bacc
from concourse.bass_utils import run_bass_kernel_spmd

dt = mybir.dt
AF = mybir.ActivationFunctionType

# problem shape (hardcoded per contract)
BFULL, TFULL = 2048, 128
S, NL, IN, OUT, H = 16, 128, 32, 8, 64
DT = 0.01
N_CORES = 8
N_ITERS = 6    # fixed-point tanh evaluations per timestep (rel_l2 ~2.3e-3 vs 2e-2 gate)
VG = 4         # value-MLP timestep group: 2 ts stacked on partitions x 2 ts on free dim


def build_kernel(T=TFULL, B=BFULL // N_CORES, n_iters=N_ITERS):
    nc = bacc.Bacc(None, target_bir_lowering=False)
    f32, f16 = dt.float32, dt.float16
    C = B // 2  # batch chunk width

    obsT16 = nc.dram_tensor("obsT16", [T, IN, B], f16, kind="ExternalInput")
    x0T = nc.dram_tensor("x0T", [S, B], f32, kind="ExternalInput")
    Wdvw = nc.dram_tensor("Wdvw", [NL, NL], f16, kind="ExternalInput")
    Wcd = nc.dram_tensor("Wcd", [S + IN, NL], f16, kind="ExternalInput")
    Wu = nc.dram_tensor("Wu", [S + IN, OUT], f16, kind="ExternalInput")
    Wuw = nc.dram_tensor("Wuw", [NL, OUT], f16, kind="ExternalInput")
    Wx = nc.dram_tensor("Wx", [S + IN, S], f16, kind="ExternalInput")
    Wxw = nc.dram_tensor("Wxw", [NL, S], f16, kind="ExternalInput")
    Wv0 = nc.dram_tensor("Wv0", [2 * IN, 2 * H], f16, kind="ExternalInput")
    Wv1 = nc.dram_tensor("Wv1", [2 * H, 2 * H], f16, kind="ExternalInput")
    Wv2 = nc.dram_tensor("Wv2", [2 * H, 2], f16, kind="ExternalInput")
    b0v = nc.dram_tensor("b0v", [NL, 1], f32, kind="ExternalInput")
    b1v = nc.dram_tensor("b1v", [NL, 1], f32, kind="ExternalInput")

    u_out = nc.dram_tensor("u_out", [T, OUT, B], f32, kind="ExternalOutput")
    v_out = nc.dram_tensor("v_out", [T, B], f32, kind="ExternalOutput")

    NV = VG * B // 2   # value-MLP free dim (half the group sits on partitions 64:128)

    with tile.TileContext(nc) as tc:
        with tc.tile_pool(name="wts", bufs=1) as wts, \
             tc.tile_pool(name="xyp", bufs=3) as xyp, \
             tc.tile_pool(name="wp", bufs=2) as wp, \
             tc.tile_pool(name="iop", bufs=3) as iop, \
             tc.tile_pool(name="vp", bufs=2) as vp, \
             tc.tile_pool(name="pw0", bufs=2, space="PSUM") as pwp0, \
             tc.tile_pool(name="pw1", bufs=2, space="PSUM") as pwp1, \
             tc.tile_pool(name="pxp0", bufs=1, space="PSUM") as pxp0, \
             tc.tile_pool(name="pxp1", bufs=1, space="PSUM") as pxp1, \
             tc.tile_pool(name="pup", bufs=1, space="PSUM") as pup, \
             tc.tile_pool(name="phh", bufs=1, space="PSUM") as php:
            pwp = [pwp0, pwp1]

            def wt(name, dram, shape, dtp):
                tl = wts.tile(shape, dtp, name=name)
                nc.sync.dma_start(tl[:], dram[:])
                return tl
            wdvw = wt("wdvw", Wdvw, [NL, NL], f16)
            wcd = wt("wcd", Wcd, [S + IN, NL], f16)
            wu = wt("wu", Wu, [S + IN, OUT], f16)
            wuw = wt("wuw", Wuw, [NL, OUT], f16)
            wx = wt("wx", Wx, [S + IN, S], f16)
            wxw = wt("wxw", Wxw, [NL, S], f16)
            wv0 = wt("wv0", Wv0, [2 * IN, 2 * H], f16)
            wv1 = wt("wv1", Wv1, [2 * H, 2 * H], f16)
            wv2 = wt("wv2", Wv2, [2 * H, 2], f16)
            b0 = wt("b0", b0v, [NL, 1], f32)
            b1 = wt("b1", b1v, [NL, 1], f32)

            # xy_h [48,B] f16: rows 0:32 = y^T, rows 32:48 = x^T; xt_r = fp32 x accum
            yst_h = iop.tile([IN, B], f16, name="yst_h0", tag="yst_h")
            nc.sync.dma_start(yst_h[:], obsT16[0])
            xt_r = xyp.tile([S, B], f32, name="xt_r0", tag="xt_r")
            nc.sync.dma_start(xt_r[:], x0T[:])
            xy_h = xyp.tile([S + IN, B], f16, name="xy_h0", tag="xy_h")
            nc.vector.tensor_copy(xy_h[0:IN, :], yst_h[:])
            nc.vector.tensor_copy(xy_h[IN:, :], xt_r[:])

            for t in range(T):
                # ---------- value MLP (grouped over VG timesteps) ----------
                if t % VG == 0:
                    with nc.named_scope(f"value_{t}"):
                        # [2*IN, 2*B]: ts t,t+1 stacked on partitions; t+2,t+3 on free dim
                        obs_v = vp.tile([2 * IN, NV], f16, name=f"obs_v{t}", tag="obs_v")
                        osrc = obsT16[t:t + VG].rearrange("(f p) k b -> (p k) (f b)", p=2)
                        nc.sync.dma_start(obs_v[:], osrc)
                        ph = php.tile([2 * H, NV], dt.float32, name=f"ph1_{t}", tag="ph")
                        nc.tensor.matmul(ph[:], wv0[:], obs_v[:], start=True, stop=True)
                        h1 = vp.tile([2 * H, NV], f16, name=f"h1_{t}", tag="h1")
                        nc.scalar.activation(h1[:], ph[:], AF.Tanh, bias=b0[:])
                        ph2 = php.tile([2 * H, NV], dt.float32, name=f"ph2_{t}", tag="ph")
                        nc.tensor.matmul(ph2[:], wv1[:], h1[:], start=True, stop=True)
                        h2 = vp.tile([2 * H, NV], f16, name=f"h2_{t}", tag="h1")
                        nc.scalar.activation(h2[:], ph2[:], AF.Tanh, bias=b1[:])
                        pv = php.tile([2, NV], dt.float32, name=f"pv{t}", tag="ph")
                        nc.tensor.matmul(pv[:], wv2[:], h2[:], start=True, stop=True)
                        v_sb = vp.tile([2, NV], f32, name=f"v_sb{t}", tag="v_sb")
                        nc.vector.tensor_copy(v_sb[:], pv[:])
                        nc.sync.dma_start(
                            v_out[t:t + VG].rearrange("(f p) b -> p (f b)", p=2), v_sb[:])

                # ---------- fixed point, 2-chunk ping-pong ----------
                with nc.named_scope(f"fp_{t}"):
                    if t < T - 1:
                        # prefetch next y into the next xy tile
                        yst_h = iop.tile([IN, B], f16, name=f"ysth{t + 1}", tag="yst_h")
                        nc.sync.dma_start(yst_h[:], obsT16[t + 1])
                        xy_hn = xyp.tile([S + IN, B], f16, name=f"xyh{t + 1}", tag="xy_h")
                        nc.vector.tensor_copy(xy_hn[0:IN, :], yst_h[:])
                    w16 = [None, None]
                    for it in range(n_iters):
                        for c in range(2):
                            cs = slice(c * C, (c + 1) * C)
                            p = pwp[c].tile([NL, C], dt.float32, name=f"pw{t}_{it}_{c}", tag=f"pw{c}")
                            if it == 0:
                                nc.tensor.matmul(p[:], wcd[:], xy_h[:, cs], start=True, stop=True)
                            else:
                                nc.tensor.matmul(p[:], wcd[:], xy_h[:, cs], start=True, stop=False)
                                nc.tensor.matmul(p[:], wdvw[:], w16[c][:], start=False, stop=True)
                            wn = wp.tile([NL, C], f16, name=f"w{t}_{it}_{c}", tag=f"w{c}")
                            nc.scalar.activation(wn[:], p[:], AF.Tanh)
                            w16[c] = wn

                # ---------- x_next (critical path), then u ----------
                with nc.named_scope(f"out_{t}"):
                    if t < T - 1:
                        pxp = [pxp0, pxp1]
                        pxc = []
                        for c in range(2):
                            cs = slice(c * C, (c + 1) * C)
                            px = pxp[c].tile([S, C], dt.float32, name=f"px{t}_{c}", tag=f"px{c}")
                            nc.tensor.matmul(px[:], wx[:], xy_h[:, cs], start=True, stop=False)
                            nc.tensor.matmul(px[:], wxw[:], w16[c][:], start=False, stop=True)
                            # critical: fp16 x for the next step's const folds
                            nc.vector.tensor_add(xy_hn[IN:, cs], px[:], xt_r[:, cs])
                            pxc.append(px)
                        # off-critical: fp32 x accumulator
                        xt_rn = xyp.tile([S, B], f32, name=f"xtr{t + 1}", tag="xt_r")
                        for c in range(2):
                            cs = slice(c * C, (c + 1) * C)
                            nc.vector.tensor_add(xt_rn[:, cs], pxc[c][:], xt_r[:, cs])

                    pu = pup.tile([OUT, B], dt.float32, name=f"pu{t}", tag="pu")
                    nc.tensor.matmul(pu[:], wu[:], xy_h[:], start=True, stop=False)
                    for c in range(2):
                        cs = slice(c * C, (c + 1) * C)
                        nc.tensor.matmul(pu[:, cs], wuw[:], w16[c][:], start=False, stop=True)
                    u_sb = iop.tile([OUT, B], f32, name=f"u_sb{t}", tag="u_sb")
                    nc.vector.tensor_copy(u_sb[:], pu[:])
                    nc.sync.dma_start(u_out[t], u_sb[:])

                    if t < T - 1:
                        xt_r, xy_h = xt_rn, xy_hn

    nc.compile()
    return nc


def host_inputs(inputs, core, n_cores=N_CORES):
    BL = inputs["obs"].shape[0] // n_cores
    sl = slice(core * BL, (core + 1) * BL)
    obs = np.ascontiguousarray(np.asarray(inputs["obs"])[sl].transpose(1, 2, 0))
    x0T = np.ascontiguousarray(np.asarray(inputs["x0"])[sl].T)
    g = lambda k: np.asarray(inputs[k])
    return {
        "obsT16": obs.astype(np.float16),
        "x0T": x0T.astype(np.float32),
        "Wdvw": g("Dvw_T").astype(np.float16),
        "Wcd": np.concatenate([g("Dvy_T"), g("Cv_T")], 0).astype(np.float16),
        "Wu": np.concatenate([g("Duy_T"), g("Cu_T")], 0).astype(np.float16),
        "Wuw": g("Duw_T").astype(np.float16),
        "Wx": np.concatenate([DT * g("By_T"), DT * g("A_T")], 0).astype(np.float16),
        "Wxw": (DT * g("Bw_T")).astype(np.float16),
        "Wv0": g("W0").astype(np.float16),
        "Wv1": np.tile(g("W1"), (2, 1)).astype(np.float16),
        "Wv2": np.tile(g("W2"), (2, 1)).astype(np.float16),
        "b0v": np.tile(g("b0").reshape(H, 1), (2, 1)).astype(np.float32),
        "b1v": np.tile(g("b1").reshape(H, 1), (2, 1)).astype(np.float32),
    }


def assemble_output(results, inputs, n_cores=N_CORES):
    obs = np.asarray(inputs["obs"])
    Bfull, T = obs.shape[0], obs.shape[1]
    BL = Bfull // n_cores
    out = np.empty((Bfull, T, 2 * OUT + 1), np.float32)
    log_stds = np.asarray(inputs["log_stds"], np.float32)
    b2 = np.asarray(inputs["b2"], np.float32)
    for c in range(n_cores):
        sl = slice(c * BL, (c + 1) * BL)
        out[sl, :, :OUT] = results[c]["u_out"].transpose(2, 0, 1)
        out[sl, :, OUT:2 * OUT] = log_stds
        out[sl, :, 2 * OUT:] = results[c]["v_out"].T[:, :, None] + b2
    return out


_NC_CACHE = {}


def _get_nc(T):
    if T not in _NC_CACHE:
        _NC_CACHE[T] = build_kernel(T=T)
    return _NC_CACHE[T]


def run_on_hw(inputs, trace=False):
    """Run the SPMD kernel; returns (full_output, exec_time_ns_or_None)."""
    T = np.asarray(inputs["obs"]).shape[1]
    nc = _get_nc(T)
    in_maps = [host_inputs(inputs, c) for c in range(N_CORES)]
    last_err = None
    for attempt in range(3):
        try:
            res = run_bass_kernel_spmd(nc, in_maps, list(range(N_CORES)), trace=trace)
            return assemble_output(res.results, inputs), res.exec_time_ns
        except Exception as e:  # transient device failures: retry
            last_err = e
    raise last_err


def kernel(**inputs) -> np.ndarray:
    out, _ = run_on_hw(inputs, trace=False)
    return out



# revision 11
# speedup vs baseline: 1.2244x; 1.2244x over previous
"""DissipativeThetaRINN Trainium2 (Bass/Tile) kernel — 8-core data parallel.

Strategy (pure data parallel, per sharding hint):
  - Batch B=2048 is split across 8 NeuronCores (256 rows/core); the tiny
    controller matrices and value-MLP weights are replicated.
  - On-device layout is transposed: features on SBUF partitions, batch on
    the free dimension (one full-width FD=256 chunk per core).
  - Two timesteps are software-pipelined ("wavefront"): timestep t+1's
    state x_{t+1} is launched from the k_early-th fixed-point iterate of
    timestep t (forward-Euler increment is DT-damped, so the early iterate
    is accurate enough), letting t+1's early iterations overlap t's late
    iterations.  Emission is slot-scheduled: timestep t occupies slots
    [3t, 3t+n_iters) and each slot carries one iteration of each of the
    two in-flight timesteps.
  - The implicit layer w = tanh(Cv x + Dvy y + Dvw w) is iterated in
    DELTA form: the pre-activation P lives in a persistent PSUM bank per
    in-flight timestep; each iteration accumulates Dvw^T @ (w_i - w_{i-1})
    with a single matmul (PSUM accumulation provides the "+ const" for
    free), and ScalarE reads the bank for the next tanh.  This halves PE
    matmul work vs re-folding the constant every iteration.
  - Matmuls run in fp16 (PSUM accumulates fp32); the x recurrence keeps an
    fp32 accumulator on device, and DT is pre-folded into the recurrence
    weights so fp16 rounding only touches the 0.01-scaled increment.
  - The value MLP (independent of the recurrence) is computed in groups of
    4 timesteps with 2 timesteps stacked on partitions (block-diagonal
    weights) and 2 on the free dim, so its matmuls/tanh use all 128
    partitions at FD=512; its three stages are spread over 3 slots.
  - log_stds broadcast and the +b2 value bias are applied host-side during
    output assembly.
  - n_iters=6 fixed-point tanh evaluations reproduce the reference's
    30-iteration result to rel_l2 ~3.3e-3 (gate is 2e-2); the iteration
    contracts at ~0.47/step and the CPU bit-accurate sim matches HW to
    4 digits.
"""
import numpy as np
import concourse.bass as bass
import concourse.mybir as mybir
import concourse.tile as tile
from concourse import bacc
from concourse.bass_utils import run_bass_kernel_spmd

dt = mybir.dt
AF = mybir.ActivationFunctionType

# problem shape (hardcoded per contract)
BFULL, TFULL = 2048, 128
S, NL, IN, OUT, H = 16, 128, 32, 8, 64
DT = 0.01
N_CORES = 8
N_ITERS = 6    # fixed-point tanh evaluations per timestep
K_EARLY = 2    # iterate used for the forward-Euler x update (early launch)
SP = 3         # slot spacing between consecutive timesteps
VG = 4         # value-MLP timestep group: 2 ts on partitions x 2 ts on free dim


def build_kernel(T=TFULL, B=BFULL // N_CORES, n_iters=N_ITERS, k_early=K_EARLY):
    nc = bacc.Bacc(None, target_bir_lowering=False)
    f32, f16 = dt.float32, dt.float16

    obsT16 = nc.dram_tensor("obsT16", [T, IN, B], f16, kind="ExternalInput")
    x0T = nc.dram_tensor("x0T", [S, B], f32, kind="ExternalInput")
    Wdvw = nc.dram_tensor("Wdvw", [NL, NL], f16, kind="ExternalInput")
    Wcd = nc.dram_tensor("Wcd", [S + IN, NL], f16, kind="ExternalInput")
    Wu = nc.dram_tensor("Wu", [S + IN, OUT], f16, kind="ExternalInput")
    Wuw = nc.dram_tensor("Wuw", [NL, OUT], f16, kind="ExternalInput")
    Wx = nc.dram_tensor("Wx", [S + IN, S], f16, kind="ExternalInput")
    Wxw = nc.dram_tensor("Wxw", [NL, S], f16, kind="ExternalInput")
    Wv0 = nc.dram_tensor("Wv0", [2 * IN, 2 * H], f16, kind="ExternalInput")
    Wv1 = nc.dram_tensor("Wv1", [2 * H, 2 * H], f16, kind="ExternalInput")
    Wv2 = nc.dram_tensor("Wv2", [2 * H, 2], f16, kind="ExternalInput")
    b0v = nc.dram_tensor("b0v", [NL, 1], f32, kind="ExternalInput")
    b1v = nc.dram_tensor("b1v", [NL, 1], f32, kind="ExternalInput")

    u_out = nc.dram_tensor("u_out", [T, OUT, B], f32, kind="ExternalOutput")
    v_out = nc.dram_tensor("v_out", [T, B], f32, kind="ExternalOutput")

    NV = VG * B // 2   # value-MLP free dim (half the group sits on partitions 64:128)

    with tile.TileContext(nc) as tc:
        with tc.tile_pool(name="wts", bufs=1) as wts, \
             tc.tile_pool(name="xyp", bufs=3) as xyp, \
             tc.tile_pool(name="xrp", bufs=2) as xrp, \
             tc.tile_pool(name="wp", bufs=3) as wp, \
             tc.tile_pool(name="dp", bufs=2) as dpp, \
             tc.tile_pool(name="iop", bufs=3) as iop, \
             tc.tile_pool(name="vp", bufs=2) as vp, \
             tc.tile_pool(name="pw0", bufs=1, space="PSUM") as pwp0, \
             tc.tile_pool(name="pw1", bufs=1, space="PSUM") as pwp1, \
             tc.tile_pool(name="pxp", bufs=1, space="PSUM") as pxp, \
             tc.tile_pool(name="pup", bufs=1, space="PSUM") as pup, \
             tc.tile_pool(name="phh", bufs=1, space="PSUM") as php:
            pwp = [pwp0, pwp1]

            def wt(name, dram, shape, dtp):
                tl = wts.tile(shape, dtp, name=name)
                nc.sync.dma_start(tl[:], dram[:])
                return tl
            wdvw = wt("wdvw", Wdvw, [NL, NL], f16)
            wcd = wt("wcd", Wcd, [S + IN, NL], f16)
            wu = wt("wu", Wu, [S + IN, OUT], f16)
            wuw = wt("wuw", Wuw, [NL, OUT], f16)
            wx = wt("wx", Wx, [S + IN, S], f16)
            wxw = wt("wxw", Wxw, [NL, S], f16)
            wv0 = wt("wv0", Wv0, [2 * IN, 2 * H], f16)
            wv1 = wt("wv1", Wv1, [2 * H, 2 * H], f16)
            wv2 = wt("wv2", Wv2, [2 * H, 2], f16)
            b0 = wt("b0", b0v, [NL, 1], f32)
            b1 = wt("b1", b1v, [NL, 1], f32)

            # initial state: xy rows 0:IN = y^T, rows IN: = x^T (f16); fp32 x accum
            xt0 = xrp.tile([S, B], f32, name="xtr0", tag="xt_r")
            nc.sync.dma_start(xt0[:], x0T[:])
            xy0 = xyp.tile([S + IN, B], f16, name="xy0", tag="xy")
            nc.sync.dma_start(xy0[0:IN, :], obsT16[0])
            nc.vector.tensor_copy(xy0[IN:, :], xt0[:])

            xy_of = {0: xy0}
            xt_of = {0: xt0}
            p_of = {}       # t -> persistent PSUM pre-activation tile
            w_of = {}       # t -> previous tanh output (for the delta sub)
            d_of = {}       # t -> delta tile feeding the next wdvw matmul
            vstate = {}

            # value stages: group g stage j emitted at slot SP*g*VG + j
            vsched = {}
            for g in range(T // VG):
                for j in range(3):
                    vsched.setdefault(SP * g * VG + j, []).append((g, j))

            n_slots = SP * (T - 1) + n_iters
            for s in range(n_slots):
                active = [(t, s - SP * t) for t in range(T)
                          if 0 <= s - SP * t < n_iters]

                # -- phase A: PE matmuls (delta accumulation) --
                for t, i in active:
                    if i == 0:
                        p = pwp[t % 2].tile([NL, B], dt.float32,
                                            name=f"pw{t}", tag="pw")
                        p_of[t] = p
                        nc.tensor.matmul(p[:], wcd[:], xy_of[t][:],
                                         start=True, stop=False)
                    else:
                        nc.tensor.matmul(p_of[t][:], wdvw[:], d_of[t][:],
                                         start=False, stop=(i == n_iters - 1))

                # -- phase B: tanh --
                wn_of = {}
                for t, i in active:
                    wn = wp.tile([NL, B], f16, name=f"w{t}_{i}", tag="w")
                    nc.scalar.activation(wn[:], p_of[t][:], AF.Tanh)
                    wn_of[t] = wn

                # -- phase C: delta for next iteration --
                for t, i in active:
                    wn = wn_of[t]
                    if i < n_iters - 1:
                        if i == 0:
                            d_of[t] = wn            # delta_1 = w_1 - 0
                        else:
                            dn = dpp.tile([NL, B], f16, name=f"d{t}_{i}", tag="d")
                            nc.vector.tensor_sub(dn[:], wn[:], w_of[t][:])
                            d_of[t] = dn
                    w_of[t] = wn

                # -- phase D: x launch / u output --
                for t, i in active:
                    wn = wn_of[t]
                    if i == k_early - 1 and t < T - 1:
                        with nc.named_scope(f"xnext_{t}"):
                            px = pxp.tile([S, B], dt.float32, name=f"px{t}", tag="px")
                            nc.tensor.matmul(px[:], wx[:], xy_of[t][:],
                                             start=True, stop=False)
                            nc.tensor.matmul(px[:], wxw[:], wn[:],
                                             start=False, stop=True)
                            xy_n = xyp.tile([S + IN, B], f16,
                                            name=f"xy{t + 1}", tag="xy")
                            nc.sync.dma_start(xy_n[0:IN, :], obsT16[t + 1])
                            nc.vector.tensor_add(xy_n[IN:, :], px[:], xt_of[t][:])
                            xtn = xrp.tile([S, B], f32, name=f"xtr{t + 1}", tag="xt_r")
                            nc.vector.tensor_add(xtn[:], px[:], xt_of[t][:])
                            xy_of[t + 1] = xy_n
                            xt_of[t + 1] = xtn
                    if i == n_iters - 1:
                        with nc.named_scope(f"uout_{t}"):
                            pu = pup.tile([OUT, B], dt.float32, name=f"pu{t}", tag="pu")
                            nc.tensor.matmul(pu[:], wu[:], xy_of[t][:],
                                             start=True, stop=False)
                            nc.tensor.matmul(pu[:], wuw[:], wn[:],
                                             start=False, stop=True)
                            u_sb = iop.tile([OUT, B], f32, name=f"u{t}", tag="u_sb")
                            nc.vector.tensor_copy(u_sb[:], pu[:])
                            nc.sync.dma_start(u_out[t], u_sb[:])
                        xy_of.pop(t, None)
                        xt_of.pop(t, None)
                        p_of.pop(t, None)
                        w_of.pop(t, None)
                        d_of.pop(t, None)

                # -- phase E: value-MLP stages --
                for g, j in vsched.get(s, ()):
                    t0 = g * VG
                    with nc.named_scope(f"value_{t0}_{j}"):
                        if j == 0:
                            obs_v = vp.tile([2 * IN, NV], f16,
                                            name=f"obsv{g}", tag="obs_v")
                            osrc = obsT16[t0:t0 + VG].rearrange(
                                "(f p) k b -> (p k) f b", p=2)
                            nc.sync.dma_start(
                                obs_v[:].rearrange("q (f b) -> q f b", f=2), osrc)
                            ph = php.tile([2 * H, NV], dt.float32,
                                          name=f"ph{g}", tag="ph")
                            nc.tensor.matmul(ph[:], wv0[:], obs_v[:],
                                             start=True, stop=True)
                            h1 = vp.tile([2 * H, NV], f16, name=f"h1{g}", tag="h1")
                            nc.scalar.activation(h1[:], ph[:], AF.Tanh, bias=b0[:])
                            vstate[g] = h1
                        elif j == 1:
                            ph2 = php.tile([2 * H, NV], dt.float32,
                                           name=f"ph2{g}", tag="ph")
                            nc.tensor.matmul(ph2[:], wv1[:], vstate[g][:],
                                             start=True, stop=True)
                            h2 = vp.tile([2 * H, NV], f16, name=f"h2{g}", tag="h1")
                            nc.scalar.activation(h2[:], ph2[:], AF.Tanh, bias=b1[:])
                            vstate[g] = h2
                        else:
                            pv = php.tile([2, NV], dt.float32, name=f"pv{g}", tag="ph")
                            nc.tensor.matmul(pv[:], wv2[:], vstate[g][:],
                                             start=True, stop=True)
                            v_sb = vp.tile([2, NV], f32, name=f"v{g}", tag="v_sb")
                            nc.vector.tensor_copy(v_sb[:], pv[:])
                            nc.sync.dma_start(
                                v_out[t0:t0 + VG].rearrange("(f p) b -> p f b", p=2),
                                v_sb[:].rearrange("p (f b) -> p f b", f=2))
                            vstate.pop(g, None)

    nc.compile()
    return nc


def host_inputs(inputs, core, n_cores=N_CORES):
    BL = inputs["obs"].shape[0] // n_cores
    sl = slice(core * BL, (core + 1) * BL)
    obs = np.ascontiguousarray(np.asarray(inputs["obs"])[sl].transpose(1, 2, 0))
    x0T = np.ascontiguousarray(np.asarray(inputs["x0"])[sl].T)
    g = lambda k: np.asarray(inputs[k])
    W0b = np.zeros((2 * IN, 2 * H), np.float16)
    W0b[0:IN, 0:H] = g("W0")
    W0b[IN:, H:] = g("W0")
    W1b = np.zeros((2 * H, 2 * H), np.float16)
    W1b[0:H, 0:H] = g("W1")
    W1b[H:, H:] = g("W1")
    W2b = np.zeros((2 * H, 2), np.float16)
    W2b[0:H, 0] = g("W2")[:, 0]
    W2b[H:, 1] = g("W2")[:, 0]
    return {
        "obsT16": obs.astype(np.float16),
        "x0T": x0T.astype(np.float32),
        "Wdvw": g("Dvw_T").astype(np.float16),
        "Wcd": np.concatenate([g("Dvy_T"), g("Cv_T")], 0).astype(np.float16),
        "Wu": np.concatenate([g("Duy_T"), g("Cu_T")], 0).astype(np.float16),
        "Wuw": g("Duw_T").astype(np.float16),
        "Wx": np.concatenate([DT * g("By_T"), DT * g("A_T")], 0).astype(np.float16),
        "Wxw": (DT * g("Bw_T")).astype(np.float16),
        "Wv0": W0b,
        "Wv1": W1b,
        "Wv2": W2b,
        "b0v": np.tile(g("b0").reshape(H, 1), (2, 1)).astype(np.float32),
        "b1v": np.tile(g("b1").reshape(H, 1), (2, 1)).astype(np.float32),
    }


def assemble_output(results, inputs, n_cores=N_CORES):
    obs = np.asarray(inputs["obs"])
    Bfull, T = obs.shape[0], obs.shape[1]
    BL = Bfull // n_cores
    out = np.empty((Bfull, T, 2 * OUT + 1), np.float32)
    log_stds = np.asarray(inputs["log_stds"], np.float32)
    b2 = np.asarray(inputs["b2"], np.float32)
    for c in range(n_cores):
        sl = slice(c * BL, (c + 1) * BL)
        out[sl, :, :OUT] = results[c]["u_out"].transpose(2, 0, 1)
        out[sl, :, OUT:2 * OUT] = log_stds
        out[sl, :, 2 * OUT:] = results[c]["v_out"].T[:, :, None] + b2
    return out


_NC_CACHE = {}


def _get_nc(T):
    if T not in _NC_CACHE:
        _NC_CACHE[T] = build_kernel(T=T)
    return _NC_CACHE[T]


def run_on_hw(inputs, trace=False):
    """Run the SPMD kernel; returns (full_output, exec_time_ns_or_None)."""
    T = np.asarray(inputs["obs"]).shape[1]
    nc = _get_nc(T)
    in_maps = [host_inputs(inputs, c) for c in range(N_CORES)]
    last_err = None
    for attempt in range(3):
        try:
            res = run_bass_kernel_spmd(nc, in_maps, list(range(N_CORES)), trace=trace)
            return assemble_output(res.results, inputs), res.exec_time_ns
        except Exception as e:  # transient device failures: retry
            last_err = e
    raise last_err


def kernel(**inputs) -> np.ndarray:
    out, _ = run_on_hw(inputs, trace=False)
    return out


# revision 13
# speedup vs baseline: 1.2919x; 1.0551x over previous
"""DissipativeThetaRINN Trainium2 (Bass/Tile) kernel — 8-core data parallel.

Strategy (pure data parallel, per sharding hint):
  - Batch B=2048 is split across 8 NeuronCores (256 rows/core); the tiny
    controller matrices and value-MLP weights are replicated.
  - On-device layout is transposed: features on SBUF partitions, batch on
    the free dimension (one full-width FD=256 chunk per core).
  - Two timesteps are software-pipelined ("wavefront"): timestep t+1's
    state x_{t+1} is launched from the k_early-th fixed-point iterate of
    timestep t (forward-Euler increment is DT-damped, so the early iterate
    is accurate enough), letting t+1's early iterations overlap t's late
    iterations.  Emission is slot-scheduled: timestep t occupies slots
    [3t, 3t+n_iters) and each slot carries one iteration of each of the
    two in-flight timesteps.
  - The implicit layer w = tanh(Cv x + Dvy y + Dvw w) is iterated in
    DELTA form: the pre-activation P lives in a persistent PSUM bank per
    in-flight timestep; each iteration accumulates Dvw^T @ (w_i - w_{i-1})
    with a single matmul (PSUM accumulation provides the "+ const" for
    free), and ScalarE reads the bank for the next tanh.  This halves PE
    matmul work vs re-folding the constant every iteration.
  - Matmuls run in fp16 (PSUM accumulates fp32); the x recurrence keeps an
    fp32 accumulator on device, and DT is pre-folded into the recurrence
    weights so fp16 rounding only touches the 0.01-scaled increment.
  - The value MLP (independent of the recurrence) is computed in groups of
    4 timesteps with 2 timesteps stacked on partitions (block-diagonal
    weights) and 2 on the free dim, so its matmuls/tanh use all 128
    partitions at FD=512; its three stages are spread over 3 slots.
  - log_stds broadcast and the +b2 value bias are applied host-side during
    output assembly.
  - n_iters=6 fixed-point tanh evaluations reproduce the reference's
    30-iteration result to rel_l2 ~3.3e-3 (gate is 2e-2); the iteration
    contracts at ~0.47/step and the CPU bit-accurate sim matches HW to
    4 digits.
"""
import numpy as np
import concourse.bass as bass
import concourse.mybir as mybir
import concourse.tile as tile
from concourse import bacc
from concourse.bass_utils import run_bass_kernel_spmd

dt = mybir.dt
AF = mybir.ActivationFunctionType

# problem shape (hardcoded per contract)
BFULL, TFULL = 2048, 128
S, NL, IN, OUT, H = 16, 128, 32, 8, 64
DT = 0.01
N_CORES = 8
N_ITERS = 6    # fixed-point tanh evaluations per timestep
K_EARLY = 3    # iterate used for the forward-Euler x update (early launch);
               # k=3 puts px(t) and pu(t-1) in the same slot so their small-M
               # matmuls col-pack into concurrent PE column groups
SP = 3         # slot spacing between consecutive timesteps
VG = 4         # value-MLP timestep group: 2 ts on partitions x 2 ts on free dim


def build_kernel(T=TFULL, B=BFULL // N_CORES, n_iters=N_ITERS, k_early=K_EARLY):
    nc = bacc.Bacc(None, target_bir_lowering=False)
    f32, f16 = dt.float32, dt.float16

    obsT16 = nc.dram_tensor("obsT16", [T, IN, B], f16, kind="ExternalInput")
    x0T = nc.dram_tensor("x0T", [S, B], f32, kind="ExternalInput")
    Wdvw = nc.dram_tensor("Wdvw", [NL, NL], f16, kind="ExternalInput")
    Wcd = nc.dram_tensor("Wcd", [S + IN, NL], f16, kind="ExternalInput")
    Wu = nc.dram_tensor("Wu", [S + IN, OUT], f16, kind="ExternalInput")
    Wuw = nc.dram_tensor("Wuw", [NL, OUT], f16, kind="ExternalInput")
    Wx = nc.dram_tensor("Wx", [S + IN, S], f16, kind="ExternalInput")
    Wxw = nc.dram_tensor("Wxw", [NL, S], f16, kind="ExternalInput")
    Wv0 = nc.dram_tensor("Wv0", [2 * IN, 2 * H], f16, kind="ExternalInput")
    Wv1 = nc.dram_tensor("Wv1", [2 * H, 2 * H], f16, kind="ExternalInput")
    Wv2 = nc.dram_tensor("Wv2", [2 * H, 2], f16, kind="ExternalInput")
    b0v = nc.dram_tensor("b0v", [NL, 1], f32, kind="ExternalInput")
    b1v = nc.dram_tensor("b1v", [NL, 1], f32, kind="ExternalInput")

    u_out = nc.dram_tensor("u_out", [T, OUT, B], f32, kind="ExternalOutput")
    v_out = nc.dram_tensor("v_out", [T, B], f32, kind="ExternalOutput")

    NV = VG * B // 2   # value-MLP free dim (half the group sits on partitions 64:128)

    with tile.TileContext(nc) as tc:
        with tc.tile_pool(name="wts", bufs=1) as wts, \
             tc.tile_pool(name="xyp", bufs=3) as xyp, \
             tc.tile_pool(name="xrp", bufs=2) as xrp, \
             tc.tile_pool(name="wp", bufs=3) as wp, \
             tc.tile_pool(name="dp", bufs=2) as dpp, \
             tc.tile_pool(name="iop", bufs=3) as iop, \
             tc.tile_pool(name="vp", bufs=2) as vp, \
             tc.tile_pool(name="pw0", bufs=1, space="PSUM") as pwp0, \
             tc.tile_pool(name="pw1", bufs=1, space="PSUM") as pwp1, \
             tc.tile_pool(name="pxp", bufs=1, space="PSUM") as pxp, \
             tc.tile_pool(name="pup", bufs=1, space="PSUM") as pup, \
             tc.tile_pool(name="phh", bufs=1, space="PSUM") as php:
            pwp = [pwp0, pwp1]

            def wt(name, dram, shape, dtp):
                tl = wts.tile(shape, dtp, name=name)
                nc.sync.dma_start(tl[:], dram[:])
                return tl
            wdvw = wt("wdvw", Wdvw, [NL, NL], f16)
            wcd = wt("wcd", Wcd, [S + IN, NL], f16)
            wu = wt("wu", Wu, [S + IN, OUT], f16)
            wuw = wt("wuw", Wuw, [NL, OUT], f16)
            wx = wt("wx", Wx, [S + IN, S], f16)
            wxw = wt("wxw", Wxw, [NL, S], f16)
            wv0 = wt("wv0", Wv0, [2 * IN, 2 * H], f16)
            wv1 = wt("wv1", Wv1, [2 * H, 2 * H], f16)
            wv2 = wt("wv2", Wv2, [2 * H, 2], f16)
            b0 = wt("b0", b0v, [NL, 1], f32)
            b1 = wt("b1", b1v, [NL, 1], f32)

            # initial state: xy rows 0:IN = y^T, rows IN: = x^T (f16); fp32 x accum
            xt0 = xrp.tile([S, B], f32, name="xtr0", tag="xt_r")
            nc.sync.dma_start(xt0[:], x0T[:])
            xy0 = xyp.tile([S + IN, B], f16, name="xy0", tag="xy")
            nc.sync.dma_start(xy0[0:IN, :], obsT16[0])
            nc.vector.tensor_copy(xy0[IN:, :], xt0[:])

            xy_of = {0: xy0}
            xt_of = {0: xt0}
            p_of = {}       # t -> persistent PSUM pre-activation tile
            w_of = {}       # t -> previous tanh output (for the delta sub)
            d_of = {}       # t -> delta tile feeding the next wdvw matmul
            vstate = {}

            # value stages: group g stage j emitted at slot SP*g*VG + j
            vsched = {}
            for g in range(T // VG):
                for j in range(3):
                    vsched.setdefault(SP * g * VG + j, []).append((g, j))

            n_slots = SP * (T - 1) + n_iters
            for s in range(n_slots):
                active = [(t, s - SP * t) for t in range(T)
                          if 0 <= s - SP * t < n_iters]

                # -- phase A: PE matmuls (delta accumulation) --
                for t, i in active:
                    if i == 0:
                        p = pwp[t % 2].tile([NL, B], dt.float32,
                                            name=f"pw{t}", tag="pw")
                        p_of[t] = p
                        nc.tensor.matmul(p[:], wcd[:], xy_of[t][:],
                                         start=True, stop=False)
                    else:
                        nc.tensor.matmul(p_of[t][:], wdvw[:], d_of[t][:],
                                         start=False, stop=(i == n_iters - 1))

                x_list = [t for t, i in active
                          if i == k_early - 1 and t < T - 1]
                u_list = [t for t, i in active if i == n_iters - 1]

                # -- phase A2: K=48 output matmuls (independent of this slot's
                # tanh) -- pu writes rows 32:40 of its bank so wu/wuw col-pack
                # into column group 1, concurrent with px's group 0.
                pu_of = {}
                px_of = {}
                for t in u_list:
                    pu = pup.tile([40, B], dt.float32, name=f"pu{t}", tag="pu")
                    pu_of[t] = pu
                    nc.tensor.matmul(pu[32:40, :], wu[:], xy_of[t][:],
                                     start=True, stop=False)
                for t in x_list:
                    px = pxp.tile([S, B], dt.float32, name=f"px{t}", tag="px")
                    px_of[t] = px
                    nc.tensor.matmul(px[:], wx[:], xy_of[t][:],
                                     start=True, stop=False)

                # -- phase B: tanh --
                wn_of = {}
                for t, i in active:
                    wn = wp.tile([NL, B], f16, name=f"w{t}_{i}", tag="w")
                    nc.scalar.activation(wn[:], p_of[t][:], AF.Tanh)
                    wn_of[t] = wn

                # -- phase C: delta for next iteration --
                for t, i in active:
                    wn = wn_of[t]
                    if i < n_iters - 1:
                        if i == 0:
                            d_of[t] = wn            # delta_1 = w_1 - 0
                        else:
                            dn = dpp.tile([NL, B], f16, name=f"d{t}_{i}", tag="d")
                            nc.vector.tensor_sub(dn[:], wn[:], w_of[t][:])
                            d_of[t] = dn
                    w_of[t] = wn

                # -- phase D: K=128 output matmuls (need this slot's tanh),
                # col-packed like phase A2, then DVE tails + DMAs --
                for t in u_list:
                    nc.tensor.matmul(pu_of[t][32:40, :], wuw[:], wn_of[t][:],
                                     start=False, stop=True)
                for t in x_list:
                    nc.tensor.matmul(px_of[t][:], wxw[:], wn_of[t][:],
                                     start=False, stop=True)
                for t in x_list:
                    with nc.named_scope(f"xnext_{t}"):
                        px = px_of[t]
                        xy_n = xyp.tile([S + IN, B], f16,
                                        name=f"xy{t + 1}", tag="xy")
                        nc.sync.dma_start(xy_n[0:IN, :], obsT16[t + 1])
                        nc.vector.tensor_add(xy_n[IN:, :], px[:], xt_of[t][:])
                        xtn = xrp.tile([S, B], f32, name=f"xtr{t + 1}", tag="xt_r")
                        nc.vector.tensor_add(xtn[:], px[:], xt_of[t][:])
                        xy_of[t + 1] = xy_n
                        xt_of[t + 1] = xtn
                for t in u_list:
                    with nc.named_scope(f"uout_{t}"):
                        u_sb = iop.tile([OUT, B], f32, name=f"u{t}", tag="u_sb")
                        nc.vector.tensor_copy(u_sb[:], pu_of[t][32:40, :])
                        nc.sync.dma_start(u_out[t], u_sb[:])
                    xy_of.pop(t, None)
                    xt_of.pop(t, None)
                    p_of.pop(t, None)
                    w_of.pop(t, None)
                    d_of.pop(t, None)

                # -- phase E: value-MLP stages --
                for g, j in vsched.get(s, ()):
                    t0 = g * VG
                    with nc.named_scope(f"value_{t0}_{j}"):
                        if j == 0:
                            obs_v = vp.tile([2 * IN, NV], f16,
                                            name=f"obsv{g}", tag="obs_v")
                            osrc = obsT16[t0:t0 + VG].rearrange(
                                "(f p) k b -> (p k) f b", p=2)
                            nc.sync.dma_start(
                                obs_v[:].rearrange("q (f b) -> q f b", f=2), osrc)
                            ph = php.tile([2 * H, NV], dt.float32,
                                          name=f"ph{g}", tag="ph")
                            nc.tensor.matmul(ph[:], wv0[:], obs_v[:],
                                             start=True, stop=True)
                            h1 = vp.tile([2 * H, NV], f16, name=f"h1{g}", tag="h1")
                            nc.scalar.activation(h1[:], ph[:], AF.Tanh, bias=b0[:])
                            vstate[g] = h1
                        elif j == 1:
                            ph2 = php.tile([2 * H, NV], dt.float32,
                                           name=f"ph2{g}", tag="ph")
                            nc.tensor.matmul(ph2[:], wv1[:], vstate[g][:],
                                             start=True, stop=True)
                            h2 = vp.tile([2 * H, NV], f16, name=f"h2{g}", tag="h1")
                            nc.scalar.activation(h2[:], ph2[:], AF.Tanh, bias=b1[:])
                            vstate[g] = h2
                        else:
                            pv = php.tile([2, NV], dt.float32, name=f"pv{g}", tag="ph")
                            nc.tensor.matmul(pv[:], wv2[:], vstate[g][:],
                                             start=True, stop=True)
                            v_sb = vp.tile([2, NV], f32, name=f"v{g}", tag="v_sb")
                            nc.vector.tensor_copy(v_sb[:], pv[:])
                            nc.sync.dma_start(
                                v_out[t0:t0 + VG].rearrange("(f p) b -> p f b", p=2),
                                v_sb[:].rearrange("p (f b) -> p f b", f=2))
                            vstate.pop(g, None)

    nc.compile()
    return nc


def host_inputs(inputs, core, n_cores=N_CORES):
    BL = inputs["obs"].shape[0] // n_cores
    sl = slice(core * BL, (core + 1) * BL)
    obs = np.ascontiguousarray(np.asarray(inputs["obs"])[sl].transpose(1, 2, 0))
    x0T = np.ascontiguousarray(np.asarray(inputs["x0"])[sl].T)
    g = lambda k: np.asarray(inputs[k])
    W0b = np.zeros((2 * IN, 2 * H), np.float16)
    W0b[0:IN, 0:H] = g("W0")
    W0b[IN:, H:] = g("W0")
    W1b = np.zeros((2 * H, 2 * H), np.float16)
    W1b[0:H, 0:H] = g("W1")
    W1b[H:, H:] = g("W1")
    W2b = np.zeros((2 * H, 2), np.float16)
    W2b[0:H, 0] = g("W2")[:, 0]
    W2b[H:, 1] = g("W2")[:, 0]
    return {
        "obsT16": obs.astype(np.float16),
        "x0T": x0T.astype(np.float32),
        "Wdvw": g("Dvw_T").astype(np.float16),
        "Wcd": np.concatenate([g("Dvy_T"), g("Cv_T")], 0).astype(np.float16),
        "Wu": np.concatenate([g("Duy_T"), g("Cu_T")], 0).astype(np.float16),
        "Wuw": g("Duw_T").astype(np.float16),
        "Wx": np.concatenate([DT * g("By_T"), DT * g("A_T")], 0).astype(np.float16),
        "Wxw": (DT * g("Bw_T")).astype(np.float16),
        "Wv0": W0b,
        "Wv1": W1b,
        "Wv2": W2b,
        "b0v": np.tile(g("b0").reshape(H, 1), (2, 1)).astype(np.float32),
        "b1v": np.tile(g("b1").reshape(H, 1), (2, 1)).astype(np.float32),
    }


def assemble_output(results, inputs, n_cores=N_CORES):
    obs = np.asarray(inputs["obs"])
    Bfull, T = obs.shape[0], obs.shape[1]
    BL = Bfull // n_cores
    out = np.empty((Bfull, T, 2 * OUT + 1), np.float32)
    log_stds = np.asarray(inputs["log_stds"], np.float32)
    b2 = np.asarray(inputs["b2"], np.float32)
    for c in range(n_cores):
        sl = slice(c * BL, (c + 1) * BL)
        out[sl, :, :OUT] = results[c]["u_out"].transpose(2, 0, 1)
        out[sl, :, OUT:2 * OUT] = log_stds
        out[sl, :, 2 * OUT:] = results[c]["v_out"].T[:, :, None] + b2
    return out


_NC_CACHE = {}


def _get_nc(T):
    if T not in _NC_CACHE:
        _NC_CACHE[T] = build_kernel(T=T)
    return _NC_CACHE[T]


def run_on_hw(inputs, trace=False):
    """Run the SPMD kernel; returns (full_output, exec_time_ns_or_None)."""
    T = np.asarray(inputs["obs"]).shape[1]
    nc = _get_nc(T)
    in_maps = [host_inputs(inputs, c) for c in range(N_CORES)]
    last_err = None
    for attempt in range(3):
        try:
            res = run_bass_kernel_spmd(nc, in_maps, list(range(N_CORES)), trace=trace)
            return assemble_output(res.results, inputs), res.exec_time_ns
        except Exception as e:  # transient device failures: retry
            last_err = e
    raise last_err


def kernel(**inputs) -> np.ndarray:
    out, _ = run_on_hw(inputs, trace=False)
    return out


# revision 22
# speedup vs baseline: 1.8691x; 1.4468x over previous
"""DissipativeThetaRINN Trainium2 (Bass/Tile) kernel — 8-core data parallel.

Strategy (pure data parallel, per sharding hint):
  - Batch B=2048 is split across 8 NeuronCores (256 rows/core); the tiny
    controller matrices and value-MLP weights are replicated.
  - On-device layout is transposed: features on SBUF partitions, batch on
    the free dimension (one full-width FD=256 chunk per core).
  - Two timesteps are software-pipelined ("wavefront"): timestep t+1's
    state x_{t+1} is launched from the k_early-th fixed-point iterate of
    timestep t (forward-Euler increment is DT-damped, so the early iterate
    is accurate enough), letting t+1's early iterations overlap t's late
    iterations.  Emission is slot-scheduled: timestep t occupies slots
    [3t, 3t+n_iters) and each slot carries one iteration of each of the
    two in-flight timesteps.
  - The implicit layer w = tanh(Cv x + Dvy y + Dvw w) is iterated in
    DELTA form: the pre-activation P lives in a persistent PSUM bank per
    in-flight timestep; each iteration accumulates Dvw^T @ (w_i - w_{i-1})
    with a single matmul (PSUM accumulation provides the "+ const" for
    free), and ScalarE reads the bank for the next tanh.  This halves PE
    matmul work vs re-folding the constant every iteration.
  - Matmuls run in fp16 (PSUM accumulates fp32); the x recurrence keeps an
    fp32 accumulator on device, and DT is pre-folded into the recurrence
    weights so fp16 rounding only touches the 0.01-scaled increment.
  - The value MLP (independent of the recurrence) is computed in groups of
    4 timesteps with 2 timesteps stacked on partitions (block-diagonal
    weights) and 2 on the free dim, so its matmuls/tanh use all 128
    partitions at FD=512; its three stages are spread over 3 slots.
  - log_stds broadcast and the +b2 value bias are applied host-side during
    output assembly.
  - n_iters=6 fixed-point tanh evaluations reproduce the reference's
    30-iteration result to rel_l2 ~3.3e-3 (gate is 2e-2); the iteration
    contracts at ~0.47/step and the CPU bit-accurate sim matches HW to
    4 digits.
"""
import numpy as np
import concourse.bass as bass
import concourse.mybir as mybir
import concourse.tile as tile
from concourse import bacc
from concourse.bass_utils import run_bass_kernel_spmd

dt = mybir.dt
AF = mybir.ActivationFunctionType

# problem shape (hardcoded per contract)
BFULL, TFULL = 2048, 128
S, NL, IN, OUT, H = 16, 128, 32, 8, 64
DT = 0.01
N_CORES = 8
N_ITERS = 5    # fixed-point tanh evaluations per timestep
K_EARLY = 2    # iterate used for the forward-Euler x update (early launch)
SP = 2         # slot spacing between consecutive timesteps -> 3 chains in
               # flight, enough independent work to hide the per-iteration
               # matmul->tanh->delta latency ring on every engine
VG = 4         # value-MLP timestep group: 2 ts on partitions x 2 ts on free dim


def build_kernel(T=TFULL, B=BFULL // N_CORES, n_iters=N_ITERS, k_early=K_EARLY):
    nc = bacc.Bacc(None, target_bir_lowering=False)
    f32, f16 = dt.float32, dt.float16

    obsT16 = nc.dram_tensor("obsT16", [T, IN, B], f16, kind="ExternalInput")
    x0T = nc.dram_tensor("x0T", [S, B], f32, kind="ExternalInput")
    Wdvw = nc.dram_tensor("Wdvw", [NL, NL], f16, kind="ExternalInput")
    Wcd = nc.dram_tensor("Wcd", [S + IN, NL], f16, kind="ExternalInput")
    # combined x/u K=48 weights: cols 0:16 = DT*[By;A], cols 32:40 = [Duy;Cu]
    Wxu = nc.dram_tensor("Wxu", [S + IN, 40], f16, kind="ExternalInput")
    Wuw = nc.dram_tensor("Wuw", [NL, OUT], f16, kind="ExternalInput")
    Wxw = nc.dram_tensor("Wxw", [NL, S], f16, kind="ExternalInput")
    Wv0 = nc.dram_tensor("Wv0", [2 * IN, 2 * H], f16, kind="ExternalInput")
    Wv1 = nc.dram_tensor("Wv1", [2 * H, 2 * H], f16, kind="ExternalInput")
    Wv2 = nc.dram_tensor("Wv2", [2 * H, 2], f16, kind="ExternalInput")
    b0v = nc.dram_tensor("b0v", [NL, 1], f32, kind="ExternalInput")
    b1v = nc.dram_tensor("b1v", [NL, 1], f32, kind="ExternalInput")

    u_out = nc.dram_tensor("u_out", [T, OUT, B], f32, kind="ExternalOutput")
    v_out = nc.dram_tensor("v_out", [T, B], f32, kind="ExternalOutput")

    NV = VG * B // 2   # value-MLP free dim (half the group sits on partitions 64:128)

    with tile.TileContext(nc) as tc:
        with tc.tile_pool(name="wts", bufs=1) as wts, \
             tc.tile_pool(name="xyp", bufs=4) as xyp, \
             tc.tile_pool(name="xrp", bufs=3) as xrp, \
             tc.tile_pool(name="wp", bufs=6) as wp, \
             tc.tile_pool(name="dp", bufs=3) as dpp, \
             tc.tile_pool(name="iop", bufs=3) as iop, \
             tc.tile_pool(name="vp", bufs=2) as vp, \
             tc.tile_pool(name="pw0", bufs=1, space="PSUM") as pwp0, \
             tc.tile_pool(name="pw1", bufs=1, space="PSUM") as pwp1, \
             tc.tile_pool(name="pw2", bufs=1, space="PSUM") as pwp2, \
             tc.tile_pool(name="pxu", bufs=2, space="PSUM") as pxup, \
             tc.tile_pool(name="phh", bufs=1, space="PSUM") as php:
            pwp = [pwp0, pwp1, pwp2]

            def wt(name, dram, shape, dtp):
                tl = wts.tile(shape, dtp, name=name)
                nc.sync.dma_start(tl[:], dram[:])
                return tl
            wdvw = wt("wdvw", Wdvw, [NL, NL], f16)
            wcd = wt("wcd", Wcd, [S + IN, NL], f16)
            wxu = wt("wxu", Wxu, [S + IN, 40], f16)
            wuw = wt("wuw", Wuw, [NL, OUT], f16)
            wxw = wt("wxw", Wxw, [NL, S], f16)
            wv0 = wt("wv0", Wv0, [2 * IN, 2 * H], f16)
            wv1 = wt("wv1", Wv1, [2 * H, 2 * H], f16)
            wv2 = wt("wv2", Wv2, [2 * H, 2], f16)
            b0 = wt("b0", b0v, [NL, 1], f32)
            b1 = wt("b1", b1v, [NL, 1], f32)

            # initial state: xy rows 0:IN = y^T, rows IN: = x^T (f16); fp32 x accum
            xt0 = xrp.tile([S, B], f32, name="xtr0", tag="xt_r")
            nc.sync.dma_start(xt0[:], x0T[:])
            xy0 = xyp.tile([S + IN, B], f16, name="xy0", tag="xy")
            nc.sync.dma_start(xy0[0:IN, :], obsT16[0])
            nc.vector.tensor_copy(xy0[IN:, :], xt0[:])

            xy_of = {0: xy0}
            xt_of = {0: xt0}
            p_of = {}       # t -> persistent PSUM pre-activation tile
            w_of = {}       # t -> previous tanh output (for the delta sub)
            d_of = {}       # t -> delta tile feeding the next wdvw matmul
            pxu_of = {}     # t -> combined x/u PSUM tile (group spans slots)
            vstate = {}

            # value stages: group g stage j emitted at slot SP*g*VG + j
            vsched = {}
            for g in range(T // VG):
                for j in range(3):
                    vsched.setdefault(SP * g * VG + j, []).append((g, j))

            n_slots = SP * (T - 1) + n_iters
            for s in range(n_slots):
                active = [(t, s - SP * t) for t in range(T)
                          if 0 <= s - SP * t < n_iters]

                # -- phase A: PE matmuls (delta accumulation) --
                for t, i in active:
                    if i == 0:
                        p = pwp[t % 3].tile([NL, B], dt.float32,
                                            name=f"pw{t}", tag="pw")
                        p_of[t] = p
                        nc.tensor.matmul(p[:], wcd[:], xy_of[t][:],
                                         start=True, stop=False)
                    else:
                        nc.tensor.matmul(p_of[t][:], wdvw[:], d_of[t][:],
                                         start=False, stop=(i == n_iters - 1))

                x_list = [t for t, i in active if i == k_early - 1]
                u_list = [t for t, i in active if i == n_iters - 1]

                # -- phase A2: combined x/u K=48 matmul (independent of this
                # slot's tanh): rows 0:16 = x increment, rows 32:40 = u --
                for t in x_list:
                    pxu = pxup.tile([40, B], dt.float32, name=f"pxu{t}", tag="pxu")
                    pxu_of[t] = pxu
                    nc.tensor.matmul(pxu[:], wxu[:], xy_of[t][:],
                                     start=True, stop=False)

                # -- phase B: tanh --
                wn_of = {}
                for t, i in active:
                    wn = wp.tile([NL, B], f16, name=f"w{t}_{i}", tag="w")
                    nc.scalar.activation(wn[:], p_of[t][:], AF.Tanh)
                    wn_of[t] = wn

                # -- phase C: delta for next iteration --
                for t, i in active:
                    wn = wn_of[t]
                    if i < n_iters - 1:
                        if i == 0:
                            d_of[t] = wn            # delta_1 = w_1 - 0
                        else:
                            dn = dpp.tile([NL, B], f16, name=f"d{t}_{i}", tag="d")
                            nc.vector.tensor_sub(dn[:], wn[:], w_of[t][:])
                            d_of[t] = dn
                    w_of[t] = wn

                # -- phase D: K=128 output matmuls (need this slot's tanh),
                # then DVE tails + DMAs --
                for t in u_list:
                    nc.tensor.matmul(pxu_of[t][32:40, :], wuw[:], wn_of[t][:],
                                     start=False, stop=True)
                for t in x_list:
                    if t < T - 1:
                        nc.tensor.matmul(pxu_of[t][0:S, :], wxw[:], wn_of[t][:],
                                         start=False, stop=False)
                for t in x_list:
                    if t < T - 1:
                        with nc.named_scope(f"xnext_{t}"):
                            pxu = pxu_of[t]
                            xy_n = xyp.tile([S + IN, B], f16,
                                            name=f"xy{t + 1}", tag="xy")
                            nc.sync.dma_start(xy_n[0:IN, :], obsT16[t + 1])
                            nc.vector.tensor_add(xy_n[IN:, :], pxu[0:S, :],
                                                 xt_of[t][:])
                            xtn = xrp.tile([S, B], f32, name=f"xtr{t + 1}",
                                           tag="xt_r")
                            nc.vector.tensor_add(xtn[:], pxu[0:S, :], xt_of[t][:])
                            xy_of[t + 1] = xy_n
                            xt_of[t + 1] = xtn
                for t in u_list:
                    with nc.named_scope(f"uout_{t}"):
                        u_sb = iop.tile([OUT, B], f32, name=f"u{t}", tag="u_sb")
                        nc.vector.tensor_copy(u_sb[:], pxu_of[t][32:40, :])
                        nc.sync.dma_start(u_out[t], u_sb[:])
                    xy_of.pop(t, None)
                    xt_of.pop(t, None)
                    p_of.pop(t, None)
                    w_of.pop(t, None)
                    d_of.pop(t, None)
                    pxu_of.pop(t, None)

                # -- phase E: value-MLP stages --
                for g, j in vsched.get(s, ()):
                    t0 = g * VG
                    with nc.named_scope(f"value_{t0}_{j}"):
                        if j == 0:
                            obs_v = vp.tile([2 * IN, NV], f16,
                                            name=f"obsv{g}", tag="obs_v")
                            osrc = obsT16[t0:t0 + VG].rearrange(
                                "(f p) k b -> (p k) f b", p=2)
                            nc.sync.dma_start(
                                obs_v[:].rearrange("q (f b) -> q f b", f=2), osrc)
                            ph = php.tile([2 * H, NV], dt.float32,
                                          name=f"ph{g}", tag="ph")
                            nc.tensor.matmul(ph[:], wv0[:], obs_v[:],
                                             start=True, stop=True)
                            h1 = vp.tile([2 * H, NV], f16, name=f"h1{g}", tag="h1")
                            nc.scalar.activation(h1[:], ph[:], AF.Tanh, bias=b0[:])
                            vstate[g] = h1
                        elif j == 1:
                            ph2 = php.tile([2 * H, NV], dt.float32,
                                           name=f"ph2{g}", tag="ph")
                            nc.tensor.matmul(ph2[:], wv1[:], vstate[g][:],
                                             start=True, stop=True)
                            h2 = vp.tile([2 * H, NV], f16, name=f"h2{g}", tag="h1")
                            nc.scalar.activation(h2[:], ph2[:], AF.Tanh, bias=b1[:])
                            vstate[g] = h2
                        else:
                            pv = php.tile([2, NV], dt.float32, name=f"pv{g}", tag="ph")
                            nc.tensor.matmul(pv[:], wv2[:], vstate[g][:],
                                             start=True, stop=True)
                            v_sb = vp.tile([2, NV], f32, name=f"v{g}", tag="v_sb")
                            nc.vector.tensor_copy(v_sb[:], pv[:])
                            nc.sync.dma_start(
                                v_out[t0:t0 + VG].rearrange("(f p) b -> p f b", p=2),
                                v_sb[:].rearrange("p (f b) -> p f b", f=2))
                            vstate.pop(g, None)

    nc.compile()
    return nc


def host_inputs(inputs, core, n_cores=N_CORES):
    BL = inputs["obs"].shape[0] // n_cores
    sl = slice(core * BL, (core + 1) * BL)
    obs = np.ascontiguousarray(np.asarray(inputs["obs"])[sl].transpose(1, 2, 0))
    x0T = np.ascontiguousarray(np.asarray(inputs["x0"])[sl].T)
    g = lambda k: np.asarray(inputs[k])
    W0b = np.zeros((2 * IN, 2 * H), np.float16)
    W0b[0:IN, 0:H] = g("W0")
    W0b[IN:, H:] = g("W0")
    W1b = np.zeros((2 * H, 2 * H), np.float16)
    W1b[0:H, 0:H] = g("W1")
    W1b[H:, H:] = g("W1")
    W2b = np.zeros((2 * H, 2), np.float16)
    W2b[0:H, 0] = g("W2")[:, 0]
    W2b[H:, 1] = g("W2")[:, 0]
    Wxu = np.zeros((S + IN, 40), np.float16)
    Wxu[:, 0:S] = np.concatenate([DT * g("By_T"), DT * g("A_T")], 0)
    Wxu[:, 32:40] = np.concatenate([g("Duy_T"), g("Cu_T")], 0)
    return {
        "obsT16": obs.astype(np.float16),
        "x0T": x0T.astype(np.float32),
        "Wdvw": g("Dvw_T").astype(np.float16),
        "Wcd": np.concatenate([g("Dvy_T"), g("Cv_T")], 0).astype(np.float16),
        "Wxu": Wxu,
        "Wuw": g("Duw_T").astype(np.float16),
        "Wxw": (DT * g("Bw_T")).astype(np.float16),
        "Wv0": W0b,
        "Wv1": W1b,
        "Wv2": W2b,
        "b0v": np.tile(g("b0").reshape(H, 1), (2, 1)).astype(np.float32),
        "b1v": np.tile(g("b1").reshape(H, 1), (2, 1)).astype(np.float32),
    }


def assemble_output(results, inputs, n_cores=N_CORES):
    obs = np.asarray(inputs["obs"])
    Bfull, T = obs.shape[0], obs.shape[1]
    BL = Bfull // n_cores
    out = np.empty((Bfull, T, 2 * OUT + 1), np.float32)
    log_stds = np.asarray(inputs["log_stds"], np.float32)
    b2 = np.asarray(inputs["b2"], np.float32)
    for c in range(n_cores):
        sl = slice(c * BL, (c + 1) * BL)
        out[sl, :, :OUT] = results[c]["u_out"].transpose(2, 0, 1)
        out[sl, :, OUT:2 * OUT] = log_stds
        out[sl, :, 2 * OUT:] = results[c]["v_out"].T[:, :, None] + b2
    return out


_NC_CACHE = {}


def _get_nc(T):
    if T not in _NC_CACHE:
        _NC_CACHE[T] = build_kernel(T=T)
    return _NC_CACHE[T]


def run_on_hw(inputs, trace=False):
    """Run the SPMD kernel; returns (full_output, exec_time_ns_or_None)."""
    T = np.asarray(inputs["obs"]).shape[1]
    nc = _get_nc(T)
    in_maps = [host_inputs(inputs, c) for c in range(N_CORES)]
    last_err = None
    for attempt in range(3):
        try:
            res = run_bass_kernel_spmd(nc, in_maps, list(range(N_CORES)), trace=trace)
            return assemble_output(res.results, inputs), res.exec_time_ns
        except Exception as e:  # transient device failures: retry
            last_err = e
    raise last_err


def kernel(**inputs) -> np.ndarray:
    out, _ = run_on_hw(inputs, trace=False)
    return out


# revision 24
# speedup vs baseline: 1.8703x; 1.0006x over previous
"""DissipativeThetaRINN Trainium2 (Bass/Tile) kernel — 8-core data parallel.

Strategy (pure data parallel, per sharding hint):
  - Batch B=2048 is split across 8 NeuronCores (256 rows/core); the tiny
    controller matrices and value-MLP weights are replicated.
  - On-device layout is transposed: features on SBUF partitions, batch on
    the free dimension (one full-width FD=256 chunk per core).
  - Two timesteps are software-pipelined ("wavefront"): timestep t+1's
    state x_{t+1} is launched from the k_early-th fixed-point iterate of
    timestep t (forward-Euler increment is DT-damped, so the early iterate
    is accurate enough), letting t+1's early iterations overlap t's late
    iterations.  Emission is slot-scheduled: timestep t occupies slots
    [SP*t, SP*t+n_iters); with SP=2, n_iters=5 up to three timesteps are
    in flight per slot, hiding the per-iteration matmul->tanh->delta
    latency ring on every engine.
  - The implicit layer w = tanh(Cv x + Dvy y + Dvw w) is iterated in
    DELTA form: the pre-activation P lives in a persistent PSUM bank per
    in-flight timestep; each iteration accumulates Dvw^T @ (w_i - w_{i-1})
    with a single matmul (PSUM accumulation provides the "+ const" for
    free), and ScalarE reads the bank for the next tanh.  This halves PE
    matmul work vs re-folding the constant every iteration.
  - Matmuls run in fp16 (PSUM accumulates fp32); the x recurrence keeps an
    fp32 accumulator on device, and DT is pre-folded into the recurrence
    weights so fp16 rounding only touches the 0.01-scaled increment.
  - The value MLP (independent of the recurrence) is computed in groups of
    4 timesteps with 2 timesteps stacked on partitions (block-diagonal
    weights) and 2 on the free dim, so its matmuls/tanh use all 128
    partitions at FD=512; its three stages are spread over 3 slots.
  - log_stds broadcast and the +b2 value bias are applied host-side during
    output assembly.
  - n_iters=6 fixed-point tanh evaluations reproduce the reference's
    30-iteration result to rel_l2 ~3.3e-3 (gate is 2e-2); the iteration
    contracts at ~0.47/step and the CPU bit-accurate sim matches HW to
    4 digits.
"""
import numpy as np
import concourse.bass as bass
import concourse.mybir as mybir
import concourse.tile as tile
from concourse import bacc
from concourse.bass_utils import run_bass_kernel_spmd

dt = mybir.dt
AF = mybir.ActivationFunctionType

# problem shape (hardcoded per contract)
BFULL, TFULL = 2048, 128
S, NL, IN, OUT, H = 16, 128, 32, 8, 64
DT = 0.01
N_CORES = 8
N_ITERS = 5    # fixed-point tanh evaluations per timestep
K_EARLY = 2    # iterate used for the forward-Euler x update (early launch)
SP = 2         # slot spacing between consecutive timesteps -> 3 chains in
               # flight, enough independent work to hide the per-iteration
               # matmul->tanh->delta latency ring on every engine
VG = 4         # value-MLP timestep group: 2 ts on partitions x 2 ts on free dim


def build_kernel(T=TFULL, B=BFULL // N_CORES, n_iters=N_ITERS, k_early=K_EARLY):
    nc = bacc.Bacc(None, target_bir_lowering=False)
    f32, f16 = dt.float32, dt.float16

    obsT16 = nc.dram_tensor("obsT16", [T, IN, B], f16, kind="ExternalInput")
    x0T = nc.dram_tensor("x0T", [S, B], f32, kind="ExternalInput")
    Wdvw = nc.dram_tensor("Wdvw", [NL, NL], f16, kind="ExternalInput")
    Wcd = nc.dram_tensor("Wcd", [S + IN, NL], f16, kind="ExternalInput")
    # combined x/u K=48 weights: cols 0:16 = DT*[By;A], cols 32:40 = [Duy;Cu]
    Wxu = nc.dram_tensor("Wxu", [S + IN, 40], f16, kind="ExternalInput")
    Wuw = nc.dram_tensor("Wuw", [NL, OUT], f16, kind="ExternalInput")
    Wxw = nc.dram_tensor("Wxw", [NL, S], f16, kind="ExternalInput")
    Wv0 = nc.dram_tensor("Wv0", [2 * IN, 2 * H], f16, kind="ExternalInput")
    Wv1 = nc.dram_tensor("Wv1", [2 * H, 2 * H], f16, kind="ExternalInput")
    Wv2 = nc.dram_tensor("Wv2", [2 * H, 2], f16, kind="ExternalInput")
    b0v = nc.dram_tensor("b0v", [NL, 1], f32, kind="ExternalInput")
    b1v = nc.dram_tensor("b1v", [NL, 1], f32, kind="ExternalInput")

    u_out = nc.dram_tensor("u_out", [T, OUT, B], f32, kind="ExternalOutput")
    v_out = nc.dram_tensor("v_out", [T, B], f32, kind="ExternalOutput")

    NV = VG * B // 2   # value-MLP free dim (half the group sits on partitions 64:128)

    with tile.TileContext(nc) as tc:
        with tc.tile_pool(name="wts", bufs=1) as wts, \
             tc.tile_pool(name="xyp", bufs=4) as xyp, \
             tc.tile_pool(name="xrp", bufs=3) as xrp, \
             tc.tile_pool(name="wp", bufs=6) as wp, \
             tc.tile_pool(name="dp", bufs=3) as dpp, \
             tc.tile_pool(name="iop", bufs=3) as iop, \
             tc.tile_pool(name="vp", bufs=2) as vp, \
             tc.tile_pool(name="pw0", bufs=1, space="PSUM") as pwp0, \
             tc.tile_pool(name="pw1", bufs=1, space="PSUM") as pwp1, \
             tc.tile_pool(name="pw2", bufs=1, space="PSUM") as pwp2, \
             tc.tile_pool(name="pxu", bufs=2, space="PSUM") as pxup, \
             tc.tile_pool(name="phh", bufs=1, space="PSUM") as php:
            pwp = [pwp0, pwp1, pwp2]

            def wt(name, dram, shape, dtp):
                tl = wts.tile(shape, dtp, name=name)
                nc.sync.dma_start(tl[:], dram[:])
                return tl
            wdvw = wt("wdvw", Wdvw, [NL, NL], f16)
            wcd = wt("wcd", Wcd, [S + IN, NL], f16)
            wxu = wt("wxu", Wxu, [S + IN, 40], f16)
            wuw = wt("wuw", Wuw, [NL, OUT], f16)
            wxw = wt("wxw", Wxw, [NL, S], f16)
            wv0 = wt("wv0", Wv0, [2 * IN, 2 * H], f16)
            wv1 = wt("wv1", Wv1, [2 * H, 2 * H], f16)
            wv2 = wt("wv2", Wv2, [2 * H, 2], f16)
            b0 = wt("b0", b0v, [NL, 1], f32)
            b1 = wt("b1", b1v, [NL, 1], f32)

            # initial state: xy rows 0:IN = y^T, rows IN: = x^T (f16); fp32 x accum
            xt0 = xrp.tile([S, B], f32, name="xtr0", tag="xt_r")
            nc.sync.dma_start(xt0[:], x0T[:])
            xy0 = xyp.tile([S + IN, B], f16, name="xy0", tag="xy")
            nc.sync.dma_start(xy0[0:IN, :], obsT16[0])
            nc.vector.tensor_copy(xy0[IN:, :], xt0[:])

            xy_of = {0: xy0}
            xt_of = {0: xt0}
            p_of = {}       # t -> persistent PSUM pre-activation tile
            w_of = {}       # t -> previous tanh output (for the delta sub)
            d_of = {}       # t -> delta tile feeding the next wdvw matmul
            pxu_of = {}     # t -> combined x/u PSUM tile (group spans slots)
            vstate = {}

            # value stages: group g stage j emitted at slot SP*g*VG + j
            vsched = {}
            for g in range(T // VG):
                for j in range(3):
                    vsched.setdefault(SP * g * VG + j, []).append((g, j))

            n_slots = SP * (T - 1) + n_iters
            for s in range(n_slots):
                active = [(t, s - SP * t) for t in range(T)
                          if 0 <= s - SP * t < n_iters]

                # -- phase A: PE matmuls (delta accumulation) --
                for t, i in active:
                    if i == 0:
                        p = pwp[t % 3].tile([NL, B], dt.float32,
                                            name=f"pw{t}", tag="pw")
                        p_of[t] = p
                        nc.tensor.matmul(p[:], wcd[:], xy_of[t][:],
                                         start=True, stop=False)
                    else:
                        nc.tensor.matmul(p_of[t][:], wdvw[:], d_of[t][:],
                                         start=False, stop=(i == n_iters - 1))

                x_list = [t for t, i in active if i == k_early - 1]
                u_list = [t for t, i in active if i == n_iters - 1]

                # -- phase A2: combined x/u K=48 matmul (independent of this
                # slot's tanh): rows 0:16 = x increment, rows 32:40 = u --
                for t in x_list:
                    pxu = pxup.tile([40, B], dt.float32, name=f"pxu{t}", tag="pxu")
                    pxu_of[t] = pxu
                    nc.tensor.matmul(pxu[:], wxu[:], xy_of[t][:],
                                     start=True, stop=False)

                # -- phase B: tanh --
                wn_of = {}
                for t, i in active:
                    wn = wp.tile([NL, B], f16, name=f"w{t}_{i}", tag="w")
                    nc.scalar.activation(wn[:], p_of[t][:], AF.Tanh)
                    wn_of[t] = wn

                # -- phase C: delta for next iteration --
                for t, i in active:
                    wn = wn_of[t]
                    if i < n_iters - 1:
                        if i == 0:
                            d_of[t] = wn            # delta_1 = w_1 - 0
                        else:
                            dn = dpp.tile([NL, B], f16, name=f"d{t}_{i}", tag="d")
                            nc.vector.tensor_sub(dn[:], wn[:], w_of[t][:])
                            d_of[t] = dn
                    w_of[t] = wn

                # -- phase D: K=128 output matmuls (need this slot's tanh),
                # then DVE tails + DMAs --
                for t in u_list:
                    nc.tensor.matmul(pxu_of[t][32:40, :], wuw[:], wn_of[t][:],
                                     start=False, stop=True)
                for t in x_list:
                    if t < T - 1:
                        nc.tensor.matmul(pxu_of[t][0:S, :], wxw[:], wn_of[t][:],
                                         start=False, stop=False)
                for t in x_list:
                    if t < T - 1:
                        with nc.named_scope(f"xnext_{t}"):
                            pxu = pxu_of[t]
                            xy_n = xyp.tile([S + IN, B], f16,
                                            name=f"xy{t + 1}", tag="xy")
                            nc.sync.dma_start(xy_n[0:IN, :], obsT16[t + 1])
                            nc.vector.tensor_add(xy_n[IN:, :], pxu[0:S, :],
                                                 xt_of[t][:])
                            xtn = xrp.tile([S, B], f32, name=f"xtr{t + 1}",
                                           tag="xt_r")
                            nc.vector.tensor_add(xtn[:], pxu[0:S, :], xt_of[t][:])
                            xy_of[t + 1] = xy_n
                            xt_of[t + 1] = xtn
                for t in u_list:
                    with nc.named_scope(f"uout_{t}"):
                        u_sb = iop.tile([OUT, B], f32, name=f"u{t}", tag="u_sb")
                        nc.vector.tensor_copy(u_sb[:], pxu_of[t][32:40, :])
                        nc.sync.dma_start(u_out[t], u_sb[:])
                    xy_of.pop(t, None)
                    xt_of.pop(t, None)
                    p_of.pop(t, None)
                    w_of.pop(t, None)
                    d_of.pop(t, None)
                    pxu_of.pop(t, None)

                # -- phase E: value-MLP stages --
                for g, j in vsched.get(s, ()):
                    t0 = g * VG
                    with nc.named_scope(f"value_{t0}_{j}"):
                        if j == 0:
                            obs_v = vp.tile([2 * IN, NV], f16,
                                            name=f"obsv{g}", tag="obs_v")
                            osrc = obsT16[t0:t0 + VG].rearrange(
                                "(f p) k b -> (p k) f b", p=2)
                            nc.sync.dma_start(
                                obs_v[:].rearrange("q (f b) -> q f b", f=2), osrc)
                            ph = php.tile([2 * H, NV], dt.float32,
                                          name=f"ph{g}", tag="ph")
                            nc.tensor.matmul(ph[:], wv0[:], obs_v[:],
                                             start=True, stop=True)
                            h1 = vp.tile([2 * H, NV], f16, name=f"h1{g}", tag="h1")
                            nc.scalar.activation(h1[:], ph[:], AF.Tanh, bias=b0[:])
                            vstate[g] = h1
                        elif j == 1:
                            ph2 = php.tile([2 * H, NV], dt.float32,
                                           name=f"ph2{g}", tag="ph")
                            nc.tensor.matmul(ph2[:], wv1[:], vstate[g][:],
                                             start=True, stop=True)
                            h2 = vp.tile([2 * H, NV], f16, name=f"h2{g}", tag="h1")
                            nc.scalar.activation(h2[:], ph2[:], AF.Tanh, bias=b1[:])
                            vstate[g] = h2
                        else:
                            pv = php.tile([2, NV], dt.float32, name=f"pv{g}", tag="ph")
                            nc.tensor.matmul(pv[:], wv2[:], vstate[g][:],
                                             start=True, stop=True)
                            v_sb = vp.tile([2, NV], f32, name=f"v{g}", tag="v_sb")
                            nc.vector.tensor_copy(v_sb[:], pv[:])
                            nc.sync.dma_start(
                                v_out[t0:t0 + VG].rearrange("(f p) b -> p f b", p=2),
                                v_sb[:].rearrange("p (f b) -> p f b", f=2))
                            vstate.pop(g, None)

    nc.compile()
    return nc


def host_inputs(inputs, core, n_cores=N_CORES):
    BL = inputs["obs"].shape[0] // n_cores
    sl = slice(core * BL, (core + 1) * BL)
    obs = np.ascontiguousarray(np.asarray(inputs["obs"])[sl].transpose(1, 2, 0))
    x0T = np.ascontiguousarray(np.asarray(inputs["x0"])[sl].T)
    g = lambda k: np.asarray(inputs[k])
    W0b = np.zeros((2 * IN, 2 * H), np.float16)
    W0b[0:IN, 0:H] = g("W0")
    W0b[IN:, H:] = g("W0")
    W1b = np.zeros((2 * H, 2 * H), np.float16)
    W1b[0:H, 0:H] = g("W1")
    W1b[H:, H:] = g("W1")
    W2b = np.zeros((2 * H, 2), np.float16)
    W2b[0:H, 0] = g("W2")[:, 0]
    W2b[H:, 1] = g("W2")[:, 0]
    Wxu = np.zeros((S + IN, 40), np.float16)
    Wxu[:, 0:S] = np.concatenate([DT * g("By_T"), DT * g("A_T")], 0)
    Wxu[:, 32:40] = np.concatenate([g("Duy_T"), g("Cu_T")], 0)
    return {
        "obsT16": obs.astype(np.float16),
        "x0T": x0T.astype(np.float32),
        "Wdvw": g("Dvw_T").astype(np.float16),
        "Wcd": np.concatenate([g("Dvy_T"), g("Cv_T")], 0).astype(np.float16),
        "Wxu": Wxu,
        "Wuw": g("Duw_T").astype(np.float16),
        "Wxw": (DT * g("Bw_T")).astype(np.float16),
        "Wv0": W0b,
        "Wv1": W1b,
        "Wv2": W2b,
        "b0v": np.tile(g("b0").reshape(H, 1), (2, 1)).astype(np.float32),
        "b1v": np.tile(g("b1").reshape(H, 1), (2, 1)).astype(np.float32),
    }


def assemble_output(results, inputs, n_cores=N_CORES):
    obs = np.asarray(inputs["obs"])
    Bfull, T = obs.shape[0], obs.shape[1]
    BL = Bfull // n_cores
    out = np.empty((Bfull, T, 2 * OUT + 1), np.float32)
    log_stds = np.asarray(inputs["log_stds"], np.float32)
    b2 = np.asarray(inputs["b2"], np.float32)
    for c in range(n_cores):
        sl = slice(c * BL, (c + 1) * BL)
        out[sl, :, :OUT] = results[c]["u_out"].transpose(2, 0, 1)
        out[sl, :, OUT:2 * OUT] = log_stds
        out[sl, :, 2 * OUT:] = results[c]["v_out"].T[:, :, None] + b2
    return out


_NC_CACHE = {}


def _get_nc(T):
    if T not in _NC_CACHE:
        _NC_CACHE[T] = build_kernel(T=T)
    return _NC_CACHE[T]


def run_on_hw(inputs, trace=False):
    """Run the SPMD kernel; returns (full_output, exec_time_ns_or_None)."""
    import time as _time
    T = np.asarray(inputs["obs"]).shape[1]
    nc = _get_nc(T)
    in_maps = [host_inputs(inputs, c) for c in range(N_CORES)]
    last_err = None
    for attempt in range(4):
        try:
            res = run_bass_kernel_spmd(nc, in_maps, list(range(N_CORES)), trace=trace)
            return assemble_output(res.results, inputs), res.exec_time_ns
        except Exception as e:  # transient device failures: retry with backoff
            last_err = e
            _time.sleep(3 * (attempt + 1))
    raise last_err


def kernel(**inputs) -> np.ndarray:
    out, _ = run_on_hw(inputs, trace=False)
    return out


# revision 27
# speedup vs baseline: 1.8795x; 1.0049x over previous
"""DissipativeThetaRINN Trainium2 (Bass/Tile) kernel — 8-core data parallel.

Strategy (pure data parallel, per sharding hint):
  - Batch B=2048 is split across 8 NeuronCores (256 rows/core); the tiny
    controller matrices and value-MLP weights are replicated.
  - On-device layout is transposed: features on SBUF partitions, batch on
    the free dimension (one full-width FD=256 chunk per core).
  - Two timesteps are software-pipelined ("wavefront"): timestep t+1's
    state x_{t+1} is launched from the k_early-th fixed-point iterate of
    timestep t (forward-Euler increment is DT-damped, so the early iterate
    is accurate enough), letting t+1's early iterations overlap t's late
    iterations.  Emission is slot-scheduled: timestep t occupies slots
    [SP*t, SP*t+n_iters); with SP=2, n_iters=5 up to three timesteps are
    in flight per slot, hiding the per-iteration matmul->tanh->delta
    latency ring on every engine.
  - The implicit layer w = tanh(Cv x + Dvy y + Dvw w) is iterated in
    DELTA form: the pre-activation P lives in a persistent PSUM bank per
    in-flight timestep; each iteration accumulates Dvw^T @ (w_i - w_{i-1})
    with a single matmul (PSUM accumulation provides the "+ const" for
    free), and ScalarE reads the bank for the next tanh.  This halves PE
    matmul work vs re-folding the constant every iteration.
  - Matmuls run in fp16 (PSUM accumulates fp32); the x recurrence keeps an
    fp32 accumulator on device, and DT is pre-folded into the recurrence
    weights so fp16 rounding only touches the 0.01-scaled increment.
  - The value MLP (independent of the recurrence) is computed in groups of
    4 timesteps with 2 timesteps stacked on partitions (block-diagonal
    weights) and 2 on the free dim, so its matmuls/tanh use all 128
    partitions at FD=512; its three stages are spread over 3 slots.
  - log_stds broadcast and the +b2 value bias are applied host-side during
    output assembly.
  - n_iters=6 fixed-point tanh evaluations reproduce the reference's
    30-iteration result to rel_l2 ~3.3e-3 (gate is 2e-2); the iteration
    contracts at ~0.47/step and the CPU bit-accurate sim matches HW to
    4 digits.
"""
import numpy as np
import concourse.bass as bass
import concourse.mybir as mybir
import concourse.tile as tile
from concourse import bacc
from concourse.bass_utils import run_bass_kernel_spmd

dt = mybir.dt
AF = mybir.ActivationFunctionType

# problem shape (hardcoded per contract)
BFULL, TFULL = 2048, 128
S, NL, IN, OUT, H = 16, 128, 32, 8, 64
DT = 0.01
N_CORES = 8
N_ITERS = 5    # fixed-point tanh evaluations per timestep
K_EARLY = 2    # iterate used for the forward-Euler x update (early launch)
SP = 2         # slot spacing between consecutive timesteps -> 3 chains in
               # flight, enough independent work to hide the per-iteration
               # matmul->tanh->delta latency ring on every engine
VG = 4         # value-MLP timestep group: 2 ts on partitions x 2 ts on free dim


def build_kernel(T=TFULL, B=BFULL // N_CORES, n_iters=N_ITERS, k_early=K_EARLY):
    nc = bacc.Bacc(None, target_bir_lowering=False)
    f32, f16 = dt.float32, dt.float16

    obsT16 = nc.dram_tensor("obsT16", [T, IN, B], f16, kind="ExternalInput")
    x0T = nc.dram_tensor("x0T", [S, B], f32, kind="ExternalInput")
    Wdvw = nc.dram_tensor("Wdvw", [NL, NL], f16, kind="ExternalInput")
    Wcd = nc.dram_tensor("Wcd", [S + IN, NL], f16, kind="ExternalInput")
    # combined x/u K=48 weights: cols 0:16 = DT*[By;A], cols 32:40 = [Duy;Cu]
    Wxu = nc.dram_tensor("Wxu", [S + IN, 40], f16, kind="ExternalInput")
    Wuw = nc.dram_tensor("Wuw", [NL, OUT], f16, kind="ExternalInput")
    Wxw = nc.dram_tensor("Wxw", [NL, S], f16, kind="ExternalInput")
    Wv0 = nc.dram_tensor("Wv0", [2 * IN, 2 * H], f16, kind="ExternalInput")
    Wv1 = nc.dram_tensor("Wv1", [2 * H, 2 * H], f16, kind="ExternalInput")
    Wv2 = nc.dram_tensor("Wv2", [2 * H, 2], f16, kind="ExternalInput")
    b0v = nc.dram_tensor("b0v", [NL, 1], f32, kind="ExternalInput")
    b1v = nc.dram_tensor("b1v", [NL, 1], f32, kind="ExternalInput")

    u_out = nc.dram_tensor("u_out", [T, OUT, B], f32, kind="ExternalOutput")
    v_out = nc.dram_tensor("v_out", [T, B], f32, kind="ExternalOutput")

    NV = VG * B // 2   # value-MLP free dim (half the group sits on partitions 64:128)

    with tile.TileContext(nc) as tc:
        with tc.tile_pool(name="wts", bufs=1) as wts, \
             tc.tile_pool(name="xyp", bufs=5) as xyp, \
             tc.tile_pool(name="xrp", bufs=4) as xrp, \
             tc.tile_pool(name="wp", bufs=8) as wp, \
             tc.tile_pool(name="dp", bufs=4) as dpp, \
             tc.tile_pool(name="iop", bufs=4) as iop, \
             tc.tile_pool(name="vp", bufs=2) as vp, \
             tc.tile_pool(name="pw0", bufs=1, space="PSUM") as pwp0, \
             tc.tile_pool(name="pw1", bufs=1, space="PSUM") as pwp1, \
             tc.tile_pool(name="pw2", bufs=1, space="PSUM") as pwp2, \
             tc.tile_pool(name="pxu", bufs=3, space="PSUM") as pxup, \
             tc.tile_pool(name="phh", bufs=1, space="PSUM") as php:
            pwp = [pwp0, pwp1, pwp2]

            def wt(name, dram, shape, dtp):
                tl = wts.tile(shape, dtp, name=name)
                nc.sync.dma_start(tl[:], dram[:])
                return tl
            wdvw = wt("wdvw", Wdvw, [NL, NL], f16)
            wcd = wt("wcd", Wcd, [S + IN, NL], f16)
            wxu = wt("wxu", Wxu, [S + IN, 40], f16)
            wuw = wt("wuw", Wuw, [NL, OUT], f16)
            wxw = wt("wxw", Wxw, [NL, S], f16)
            wv0 = wt("wv0", Wv0, [2 * IN, 2 * H], f16)
            wv1 = wt("wv1", Wv1, [2 * H, 2 * H], f16)
            wv2 = wt("wv2", Wv2, [2 * H, 2], f16)
            b0 = wt("b0", b0v, [NL, 1], f32)
            b1 = wt("b1", b1v, [NL, 1], f32)

            # initial state: xy rows 0:IN = y^T, rows IN: = x^T (f16); fp32 x accum
            xt0 = xrp.tile([S, B], f32, name="xtr0", tag="xt_r")
            nc.sync.dma_start(xt0[:], x0T[:])
            xy0 = xyp.tile([S + IN, B], f16, name="xy0", tag="xy")
            nc.sync.dma_start(xy0[0:IN, :], obsT16[0])
            nc.vector.tensor_copy(xy0[IN:, :], xt0[:])

            xy_of = {0: xy0}
            xt_of = {0: xt0}
            p_of = {}       # t -> persistent PSUM pre-activation tile
            w_of = {}       # t -> previous tanh output (for the delta sub)
            d_of = {}       # t -> delta tile feeding the next wdvw matmul
            pxu_of = {}     # t -> combined x/u PSUM tile (group spans slots)
            vstate = {}

            # value stages: group g stage j emitted at slot SP*g*VG + j
            vsched = {}
            for g in range(T // VG):
                for j in range(3):
                    vsched.setdefault(SP * g * VG + j, []).append((g, j))

            n_slots = SP * (T - 1) + n_iters
            for s in range(n_slots):
                active = [(t, s - SP * t) for t in range(T)
                          if 0 <= s - SP * t < n_iters]

                # -- phase A: PE matmuls (delta accumulation) --
                for t, i in active:
                    if i == 0:
                        if t not in p_of:   # t=0 cold start; others prefolded
                            p = pwp[t % 3].tile([NL, B], dt.float32,
                                                name=f"pw{t}", tag="pw")
                            p_of[t] = p
                            nc.tensor.matmul(p[:], wcd[:], xy_of[t][:],
                                             start=True, stop=False)
                    else:
                        nc.tensor.matmul(p_of[t][:], wdvw[:], d_of[t][:],
                                         start=False, stop=(i == n_iters - 1))

                x_list = [t for t, i in active if i == k_early - 1]
                u_list = [t for t, i in active if i == n_iters - 1]

                # -- phase A2: combined x/u K=48 matmul (independent of this
                # slot's tanh): rows 0:16 = x increment, rows 32:40 = u --
                for t in x_list:
                    pxu = pxup.tile([40, B], dt.float32, name=f"pxu{t}", tag="pxu")
                    pxu_of[t] = pxu
                    nc.tensor.matmul(pxu[:], wxu[:], xy_of[t][:],
                                     start=True, stop=False)

                # -- phase B: tanh --
                wn_of = {}
                for t, i in active:
                    wn = wp.tile([NL, B], f16, name=f"w{t}_{i}", tag="w")
                    nc.scalar.activation(wn[:], p_of[t][:], AF.Tanh)
                    wn_of[t] = wn

                # -- phase C: delta for next iteration --
                for t, i in active:
                    wn = wn_of[t]
                    if i < n_iters - 1:
                        if i == 0:
                            d_of[t] = wn            # delta_1 = w_1 - 0
                        else:
                            dn = dpp.tile([NL, B], f16, name=f"d{t}_{i}", tag="d")
                            nc.vector.tensor_sub(dn[:], wn[:], w_of[t][:])
                            d_of[t] = dn
                    w_of[t] = wn

                # -- phase D: K=128 output matmuls (need this slot's tanh),
                # then DVE tails + DMAs --
                for t in u_list:
                    nc.tensor.matmul(pxu_of[t][32:40, :], wuw[:], wn_of[t][:],
                                     start=False, stop=True)
                for t in x_list:
                    if t < T - 1:
                        nc.tensor.matmul(pxu_of[t][0:S, :], wxw[:], wn_of[t][:],
                                         start=False, stop=False)
                for t in x_list:
                    if t < T - 1:
                        with nc.named_scope(f"xnext_{t}"):
                            pxu = pxu_of[t]
                            xy_n = xyp.tile([S + IN, B], f16,
                                            name=f"xy{t + 1}", tag="xy")
                            nc.sync.dma_start(xy_n[0:IN, :], obsT16[t + 1])
                            nc.vector.tensor_add(xy_n[IN:, :], pxu[0:S, :],
                                                 xt_of[t][:])
                            xtn = xrp.tile([S, B], f32, name=f"xtr{t + 1}",
                                           tag="xt_r")
                            nc.vector.tensor_add(xtn[:], pxu[0:S, :], xt_of[t][:])
                            xy_of[t + 1] = xy_n
                            xt_of[t + 1] = xtn
                for t in u_list:
                    with nc.named_scope(f"uout_{t}"):
                        u_sb = iop.tile([OUT, B], f32, name=f"u{t}", tag="u_sb")
                        nc.vector.tensor_copy(u_sb[:], pxu_of[t][32:40, :])
                        nc.sync.dma_start(u_out[t], u_sb[:])
                    xy_of.pop(t, None)
                    xt_of.pop(t, None)
                    p_of.pop(t, None)
                    w_of.pop(t, None)
                    d_of.pop(t, None)
                    pxu_of.pop(t, None)

                # -- phase E: value-MLP stages --
                for g, j in vsched.get(s, ()):
                    t0 = g * VG
                    with nc.named_scope(f"value_{t0}_{j}"):
                        if j == 0:
                            obs_v = vp.tile([2 * IN, NV], f16,
                                            name=f"obsv{g}", tag="obs_v")
                            osrc = obsT16[t0:t0 + VG].rearrange(
                                "(f p) k b -> (p k) f b", p=2)
                            nc.sync.dma_start(
                                obs_v[:].rearrange("q (f b) -> q f b", f=2), osrc)
                            ph = php.tile([2 * H, NV], dt.float32,
                                          name=f"ph{g}", tag="ph")
                            nc.tensor.matmul(ph[:], wv0[:], obs_v[:],
                                             start=True, stop=True)
                            h1 = vp.tile([2 * H, NV], f16, name=f"h1{g}", tag="h1")
                            nc.scalar.activation(h1[:], ph[:], AF.Tanh, bias=b0[:])
                            vstate[g] = h1
                        elif j == 1:
                            ph2 = php.tile([2 * H, NV], dt.float32,
                                           name=f"ph2{g}", tag="ph")
                            nc.tensor.matmul(ph2[:], wv1[:], vstate[g][:],
                                             start=True, stop=True)
                            h2 = vp.tile([2 * H, NV], f16, name=f"h2{g}", tag="h1")
                            nc.scalar.activation(h2[:], ph2[:], AF.Tanh, bias=b1[:])
                            vstate[g] = h2
                        else:
                            pv = php.tile([2, NV], dt.float32, name=f"pv{g}", tag="ph")
                            nc.tensor.matmul(pv[:], wv2[:], vstate[g][:],
                                             start=True, stop=True)
                            v_sb = vp.tile([2, NV], f32, name=f"v{g}", tag="v_sb")
                            nc.vector.tensor_copy(v_sb[:], pv[:])
                            nc.sync.dma_start(
                                v_out[t0:t0 + VG].rearrange("(f p) b -> p f b", p=2),
                                v_sb[:].rearrange("p (f b) -> p f b", f=2))
                            vstate.pop(g, None)

                # -- phase F: prefold the chain that starts next slot (its xy
                # was just built in phase D), decoupling chain entry from its
                # first tanh --
                t_next = (s + 1) // SP
                if (s + 1) % SP == 0 and 0 < t_next < T:
                    p = pwp[t_next % 3].tile([NL, B], dt.float32,
                                             name=f"pw{t_next}", tag="pw")
                    p_of[t_next] = p
                    nc.tensor.matmul(p[:], wcd[:], xy_of[t_next][:],
                                     start=True, stop=False)

    nc.compile()
    return nc


def host_inputs(inputs, core, n_cores=N_CORES):
    BL = inputs["obs"].shape[0] // n_cores
    sl = slice(core * BL, (core + 1) * BL)
    obs = np.ascontiguousarray(np.asarray(inputs["obs"])[sl].transpose(1, 2, 0))
    x0T = np.ascontiguousarray(np.asarray(inputs["x0"])[sl].T)
    g = lambda k: np.asarray(inputs[k])
    W0b = np.zeros((2 * IN, 2 * H), np.float16)
    W0b[0:IN, 0:H] = g("W0")
    W0b[IN:, H:] = g("W0")
    W1b = np.zeros((2 * H, 2 * H), np.float16)
    W1b[0:H, 0:H] = g("W1")
    W1b[H:, H:] = g("W1")
    W2b = np.zeros((2 * H, 2), np.float16)
    W2b[0:H, 0] = g("W2")[:, 0]
    W2b[H:, 1] = g("W2")[:, 0]
    Wxu = np.zeros((S + IN, 40), np.float16)
    Wxu[:, 0:S] = np.concatenate([DT * g("By_T"), DT * g("A_T")], 0)
    Wxu[:, 32:40] = np.concatenate([g("Duy_T"), g("Cu_T")], 0)
    return {
        "obsT16": obs.astype(np.float16),
        "x0T": x0T.astype(np.float32),
        "Wdvw": g("Dvw_T").astype(np.float16),
        "Wcd": np.concatenate([g("Dvy_T"), g("Cv_T")], 0).astype(np.float16),
        "Wxu": Wxu,
        "Wuw": g("Duw_T").astype(np.float16),
        "Wxw": (DT * g("Bw_T")).astype(np.float16),
        "Wv0": W0b,
        "Wv1": W1b,
        "Wv2": W2b,
        "b0v": np.tile(g("b0").reshape(H, 1), (2, 1)).astype(np.float32),
        "b1v": np.tile(g("b1").reshape(H, 1), (2, 1)).astype(np.float32),
    }


def assemble_output(results, inputs, n_cores=N_CORES):
    obs = np.asarray(inputs["obs"])
    Bfull, T = obs.shape[0], obs.shape[1]
    BL = Bfull // n_cores
    out = np.empty((Bfull, T, 2 * OUT + 1), np.float32)
    log_stds = np.asarray(inputs["log_stds"], np.float32)
    b2 = np.asarray(inputs["b2"], np.float32)
    for c in range(n_cores):
        sl = slice(c * BL, (c + 1) * BL)
        out[sl, :, :OUT] = results[c]["u_out"].transpose(2, 0, 1)
        out[sl, :, OUT:2 * OUT] = log_stds
        out[sl, :, 2 * OUT:] = results[c]["v_out"].T[:, :, None] + b2
    return out


_NC_CACHE = {}


def _get_nc(T):
    if T not in _NC_CACHE:
        _NC_CACHE[T] = build_kernel(T=T)
    return _NC_CACHE[T]


def run_on_hw(inputs, trace=False):
    """Run the SPMD kernel; returns (full_output, exec_time_ns_or_None)."""
    import time as _time
    T = np.asarray(inputs["obs"]).shape[1]
    nc = _get_nc(T)
    in_maps = [host_inputs(inputs, c) for c in range(N_CORES)]
    last_err = None
    for attempt in range(4):
        try:
            res = run_bass_kernel_spmd(nc, in_maps, list(range(N_CORES)), trace=trace)
            return assemble_output(res.results, inputs), res.exec_time_ns
        except Exception as e:  # transient device failures: retry with backoff
            last_err = e
            _time.sleep(3 * (attempt + 1))
    raise last_err


def kernel(**inputs) -> np.ndarray:
    out, _ = run_on_hw(inputs, trace=False)
    return out
